# revision 2
# baseline (speedup 1.0000x reference)
"""LoRA linear (y = x @ (W + s*B@A)^T + bias) on 8 Trainium2 NeuronCores.

Strategy: pure data parallel over the token dim. The LoRA update is folded
into the weight on the host (W' = W + 4.0 * B @ A, rank-8 update, ~17 MFLOP
in numpy), so the device kernel is a plain linear. Operands are cast to
bf16 on the host (rel fro error ~3e-4 per operand, gate is 2e-2): this
halves input HBM traffic vs fp32 and runs the PE at the full 1 col/cycle
bf16 rate. Accumulation stays fp32 in PSUM; the output is written bf16
(another ~1e-3 rounding) and upcast on the host.

Per core: out[2048, 1024] = xT[:, shard].T @ wT + bias
  - wT [1024(d), 1024(o)] bf16 resident in SBUF (2 MiB), loaded once
  - x resident as 8 d-tiles [128, 2048] bf16 (4 MiB)
  - all input DMAs on one sequencer (sync) in exact consumption order, so
    queue completion order matches the matmul stream (DMA fill is the
    startup critical path)
  - psum [128(n), 1024(o)] accumulated over 8 d-tiles, 2 o-halves of 512;
    4 psum tiles (all 8 banks) accumulate side by side so each arriving
    (w[d], x[d]) slice enables 8 matmuls during the fill phase
  - DVE adds bias (broadcast into [128, 1024] SBUF once) on PSUM eviction,
    writing bf16
"""

import os
import sys

import numpy as np

for _p in ("/opt/trn_rl_repo", "/opt/pypackages"):
    if os.path.isdir(_p) and _p not in sys.path:
        sys.path.append(_p)

import ml_dtypes  # noqa: E402

try:
    import jax

    jax.config.update(
        "jax_compilation_cache_dir", os.path.expanduser("~/.cache/jax_bass_cache")
    )
    jax.config.update("jax_persistent_cache_min_compile_time_secs", 0.0)
except Exception:
    pass

try:
    # bass_utils imports this when tracing is requested via BASS_TRACE; the
    # agent image ships a stub antenv without it. Register a no-op fallback
    # so a trace request degrades to "no trace" instead of crashing.
    from antenv import axon_hooks as _axon_hooks  # noqa: F401
except ImportError:
    import types as _types

    import antenv as _antenv

    _hooks = _types.ModuleType("antenv.axon_hooks")
    _hooks._hook = None
    _hooks.set_axon_ntff_profile_hook = lambda h: setattr(_hooks, "_hook", h)
    _hooks.get_axon_ntff_profile_hook = lambda: _hooks._hook
    sys.modules["antenv.axon_hooks"] = _hooks
    _antenv.axon_hooks = _hooks

import concourse.bass as bass  # noqa: E402,F401
import concourse.mybir as mybir  # noqa: E402
import concourse.tile as tile  # noqa: E402
from concourse import bacc  # noqa: E402
from concourse.bass_utils import run_bass_kernel_spmd  # noqa: E402

N_CORES = 8
N_TOK, D_IN, D_OUT = 16384, 1024, 1024
N_SHARD = N_TOK // N_CORES  # 2048 tokens per core
P = 128
SCALING = 4.0  # alpha / r = 32 / 8

_CACHE: dict = {}


def build_nc():
    f32 = mybir.dt.float32
    bf16 = mybir.dt.bfloat16
    nc = bacc.Bacc("TRN2", target_bir_lowering=False, debug=False)

    xT = nc.dram_tensor("xT", [D_IN, N_SHARD], bf16, kind="ExternalInput")
    wT = nc.dram_tensor("wT", [D_IN, D_OUT], bf16, kind="ExternalInput")
    bias = nc.dram_tensor("bias", [1, D_OUT], f32, kind="ExternalInput")
    out = nc.dram_tensor("out", [N_SHARD, D_OUT], bf16, kind="ExternalOutput")

    KT = D_IN // P  # 8 contraction tiles
    NBLK = 512  # tokens per group (4 psum tiles of 128)
    GRP = NBLK // P  # 4 psum tiles accumulated concurrently (8 banks)
    OH = 512  # max fp32 moving free dim (one PSUM bank)

    NGRP = N_SHARD // NBLK
    with tile.TileContext(nc) as tc:
        with tc.tile_pool(name="const", bufs=1) as const_pool, \
                tc.tile_pool(name="op", bufs=8) as out_pool, \
                tc.tile_pool(name="ps", bufs=GRP, space="PSUM") as psum_pool:
            # DMA queues drain lines in global issue order, so ALL input DMAs
            # go on one sequencer (sync) in exact consumption order:
            # (w0,x0), (w1,x1), ..., bias, then the remaining token range.
            # The PE starts after the first ~0.5MiB pair instead of the whole
            # 6.3MiB fill.
            w_tiles = [
                const_pool.tile([P, D_OUT], bf16, name=f"w{t}")
                for t in range(KT)
            ]
            # x fully resident: 8 d-tiles x [128, 2048] bf16 = 4 MiB.
            x_tiles = [
                const_pool.tile([P, N_SHARD], bf16, name=f"x{t}")
                for t in range(KT)
            ]
            bias_sb = const_pool.tile([P, D_OUT], f32)

            # Leading warm-up matmuls (zeroed bf16 scratch, no data deps):
            # cold N=512 matmuls occupy the PE before the first real operands
            # land, pre-paying the p-state ramp / HAM clock-gate busy window.
            warm_x = const_pool.tile([P, P], bf16)
            warm_w = const_pool.tile([P, OH], bf16)
            nc.gpsimd.memset(warm_x[:], 0.0)
            nc.gpsimd.memset(warm_w[:], 0.0)
            warm_ps = psum_pool.tile([P, OH], f32, name="warm_ps", tag="psum")
            for _ in range(5):
                nc.tensor.matmul(warm_ps[:], warm_x[:], warm_w[:],
                                 start=True, stop=True)

            # Startup stream in exact consumption order. Tile dependencies
            # are sub-tile-range granular, so x loads in group-sized slices:
            # group 0 only needs tokens [0:512) of each d-slice. First pair
            # is quarter-split so matmul #1 waits on ~200KB.
            nc.sync.dma_start(w_tiles[0][:, 0:OH], wT[0:P, 0:OH])
            nc.sync.dma_start(x_tiles[0][:, 0:P], xT[0:P, 0:P])
            nc.sync.dma_start(w_tiles[0][:, OH:D_OUT], wT[0:P, OH:D_OUT])
            nc.sync.dma_start(x_tiles[0][:, P:NBLK], xT[0:P, P:NBLK])
            for t in range(1, KT):
                nc.sync.dma_start(w_tiles[t][:], wT[t * P:(t + 1) * P, :])
                nc.sync.dma_start(
                    x_tiles[t][:, 0:NBLK], xT[t * P:(t + 1) * P, 0:NBLK]
                )
            nc.sync.dma_start(bias_sb[:], bias[:].to_broadcast((P, D_OUT)))
            # Group 1 tokens per-d (fine-grained for early pipelining), then
            # groups 2+3 as one coarse 2KB-line load per d.
            for t in range(KT):
                nc.sync.dma_start(
                    x_tiles[t][:, NBLK:2 * NBLK],
                    xT[t * P:(t + 1) * P, NBLK:2 * NBLK],
                )
            for t in range(KT):
                nc.sync.dma_start(
                    x_tiles[t][:, 2 * NBLK:N_SHARD],
                    xT[t * P:(t + 1) * P, 2 * NBLK:N_SHARD],
                )

            def evict(g, i, psum, halves=False):
                n0 = g * NBLK + i * P
                o_sb = out_pool.tile([P, D_OUT], bf16)
                if halves:
                    for h in range(2):
                        sl = slice(h * OH, (h + 1) * OH)
                        nc.vector.tensor_add(o_sb[:, sl], psum[:, sl],
                                             bias_sb[:, sl])
                        nc.sync.dma_start(out[n0:n0 + P, sl], o_sb[:, sl])
                else:
                    nc.vector.tensor_add(o_sb[:], psum[:], bias_sb[:])
                    nc.sync.dma_start(out[n0:n0 + P, :], o_sb[:])

            for g in range(NGRP):
                xt = [
                    x_tiles[t][:, g * NBLK:(g + 1) * NBLK]
                    for t in range(KT)
                ]
                psums = [
                    psum_pool.tile([P, D_OUT], f32, name=f"ps_g{g}_{i}",
                                   tag="psum")
                    for i in range(GRP)
                ]
                if g < NGRP - 1:
                    # d-outer: each arriving (w[d], x[d]) slice immediately
                    # enables 8 matmuls while later slices are in flight.
                    for d in range(KT):
                        for i in range(GRP):
                            lhsT = xt[d][:, i * P:(i + 1) * P]
                            for h in range(D_OUT // OH):
                                nc.tensor.matmul(
                                    psums[i][:, h * OH:(h + 1) * OH],
                                    lhsT,
                                    w_tiles[d][:, h * OH:(h + 1) * OH],
                                    start=(d == 0),
                                    stop=(d == KT - 1),
                                )
                    for i in range(GRP):
                        evict(g, i, psums[i])
                else:
                    # last group, data resident: i-outer spreads psum
                    # completions so the tail isn't 4 serialized evictions.
                    for i in range(GRP):
                        for d in range(KT):
                            lhsT = xt[d][:, i * P:(i + 1) * P]
                            for h in range(D_OUT // OH):
                                nc.tensor.matmul(
                                    psums[i][:, h * OH:(h + 1) * OH],
                                    lhsT,
                                    w_tiles[d][:, h * OH:(h + 1) * OH],
                                    start=(d == 0),
                                    stop=(d == KT - 1),
                                )
                        evict(g, i, psums[i], halves=True)

    nc.finalize()
    return nc


def _get_nc():
    if "nc" not in _CACHE:
        _CACHE["nc"] = build_nc()
    return _CACHE["nc"]


def kernel(x, weight, bias, A, B):
    x = np.asarray(x, dtype=np.float32)
    weight = np.asarray(weight, dtype=np.float32)
    bias = np.asarray(bias, dtype=np.float32)
    A = np.asarray(A, dtype=np.float32)
    B = np.asarray(B, dtype=np.float32)

    # Fold the rank-8 LoRA update into the weight (exact up to fp32 rounding).
    w_eff = (
        weight.astype(np.float64) + SCALING * (B.astype(np.float64) @ A.astype(np.float64))
    ).astype(np.float32)
    bf16 = ml_dtypes.bfloat16
    wT = np.ascontiguousarray(w_eff.T.astype(bf16))  # [d, o]
    xT = np.ascontiguousarray(x.T.astype(bf16))  # [d, n]
    bias2d = np.ascontiguousarray(bias.reshape(1, D_OUT))

    nc = _get_nc()
    in_maps = [
        {
            "xT": np.ascontiguousarray(xT[:, c * N_SHARD:(c + 1) * N_SHARD]),
            "wT": wT,
            "bias": bias2d,
        }
        for c in range(N_CORES)
    ]
    trace_kwargs = {}
    if os.environ.get("KERNEL_TRACE") == "1":
        trace_kwargs = {"trace": True}
    res = run_bass_kernel_spmd(nc, in_maps, list(range(N_CORES)), **trace_kwargs)
    _CACHE["last_results"] = res
    return np.concatenate(
        [r["out"].astype(np.float32) for r in res.results], axis=0
    )


# revision 6
# speedup vs baseline: 1.0224x; 1.0224x over previous
"""LoRA linear (y = x @ (W + s*B@A)^T + bias) on 8 Trainium2 NeuronCores.

Strategy: pure data parallel over the token dim. The LoRA update is folded
into the weight on the host (W' = W + 4.0 * B @ A, rank-8 update), so the
device kernel is a plain linear.

Measured facts this schedule is built around (from NTFF profiles):
  - 512-col matmul cadence: 227ns with an f32r moving operand vs 259ns
    with bf16 moving. So W (the moving operand) stays f32r; x (the
    stationary operand) must match width class (BIR verifier rejects
    32-bit x 16-bit mixes), so x stays f32r too.
  - DMA descriptor issue costs ~650ns *per dma_start on the issuing
    engine*; the old all-on-sync fill was descriptor-issue-bound (17
    descriptors ~ 11us). Descriptors are now spread: W on sync, x on
    gpsimd, bias on scalar, outputs on vector (right after each eviction,
    same-engine ordering for free).
  - Output is written bf16 (host upcasts), halving output traffic and the
    end-of-kernel write burst.
  - The NC clock drops to half rate ~2.5us after the PE goes idle and the
    fixed ~285-instruction wrapper epilogue then runs at half speed;
    trailing dummy matmuls keep the PE (and clock) busy through the
    eviction/DMA drain.

Per core: out[2048, 1024] = xT[:, shard].T @ wT + bias
  - wT [1024(d), 1024(o)] f32r resident in SBUF (4 MiB), loaded once
  - x resident as 8 d-tiles [128, 2048] f32r (8 MiB)
  - psum [128(n), 1024(o)] accumulated over 8 d-tiles, 2 o-halves of 512;
    4 psum tiles (all 8 banks) accumulate side by side so each arriving
    (w[d], x[d]) slice enables 8 matmuls during the fill phase
  - DVE adds bias (broadcast into [128, 1024] SBUF once) on PSUM eviction,
    writing bf16
"""

import os
import sys

import numpy as np

for _p in ("/opt/trn_rl_repo", "/opt/pypackages"):
    if os.path.isdir(_p) and _p not in sys.path:
        sys.path.append(_p)

import ml_dtypes  # noqa: E402,F401

try:
    import jax

    jax.config.update(
        "jax_compilation_cache_dir", os.path.expanduser("~/.cache/jax_bass_cache")
    )
    jax.config.update("jax_persistent_cache_min_compile_time_secs", 0.0)
except Exception:
    pass

try:
    # bass_utils imports this when tracing is requested via BASS_TRACE; the
    # agent image ships a stub antenv without it. Register a no-op fallback
    # so a trace request degrades to "no trace" instead of crashing.
    from antenv import axon_hooks as _axon_hooks  # noqa: F401
except ImportError:
    import types as _types

    import antenv as _antenv

    _hooks = _types.ModuleType("antenv.axon_hooks")
    _hooks._hook = None
    _hooks.set_axon_ntff_profile_hook = lambda h: setattr(_hooks, "_hook", h)
    _hooks.get_axon_ntff_profile_hook = lambda: _hooks._hook
    sys.modules["antenv.axon_hooks"] = _hooks
    _antenv.axon_hooks = _hooks

import concourse.bass as bass  # noqa: E402,F401
import concourse.mybir as mybir  # noqa: E402
import concourse.tile as tile  # noqa: E402
from concourse import bacc  # noqa: E402
from concourse.bass_utils import run_bass_kernel_spmd  # noqa: E402

N_CORES = 8
N_TOK, D_IN, D_OUT = 16384, 1024, 1024
N_SHARD = N_TOK // N_CORES  # 2048 tokens per core
P = 128
SCALING = 4.0  # alpha / r = 32 / 8

_CACHE: dict = {}


def build_nc():
    f32 = mybir.dt.float32
    f32r = mybir.dt.float32r
    f16 = mybir.dt.float16
    bf16 = mybir.dt.bfloat16
    nc = bacc.Bacc("TRN2", target_bir_lowering=False, debug=False)

    xT = nc.dram_tensor("xT", [D_IN, N_SHARD], f32r, kind="ExternalInput")
    wT = nc.dram_tensor("wT", [D_IN, D_OUT], f32r, kind="ExternalInput")
    bias = nc.dram_tensor("bias", [1, D_OUT], f32, kind="ExternalInput")
    out = nc.dram_tensor("out", [N_SHARD, D_OUT], bf16, kind="ExternalOutput")

    KT = D_IN // P  # 8 contraction tiles
    NBLK = 512  # tokens per group (4 psum tiles of 128)
    GRP = NBLK // P  # 4 psum tiles accumulated concurrently (8 banks)
    OH = 512  # max fp32 moving free dim (one PSUM bank)

    NGRP = N_SHARD // NBLK
    with tile.TileContext(nc) as tc:
        with tc.tile_pool(name="const", bufs=1) as const_pool, \
                tc.tile_pool(name="op", bufs=8) as out_pool, \
                tc.tile_pool(name="ps", bufs=GRP, space="PSUM") as psum_pool:
            w_tiles = [
                const_pool.tile([P, D_OUT], f32r, name=f"w{t}")
                for t in range(KT)
            ]
            x_tiles = [
                const_pool.tile([P, N_SHARD], f32r, name=f"x{t}")
                for t in range(KT)
            ]
            bias_sb = const_pool.tile([P, D_OUT], f32)

            # W descriptors on the sync sequencer, in d consumption order.
            # First tile split in halves so matmul #1 waits on ~256KB.
            nc.sync.dma_start(w_tiles[0][:, 0:OH], wT[0:P, 0:OH])
            nc.sync.dma_start(w_tiles[0][:, OH:D_OUT], wT[0:P, OH:D_OUT])
            for t in range(1, KT):
                nc.sync.dma_start(w_tiles[t][:], wT[t * P:(t + 1) * P, :])

            # x descriptors on the gpsimd sequencer, concurrently with W:
            # group-0 token slices first (fine-grained), then the remainder.
            nc.gpsimd.dma_start(x_tiles[0][:, 0:P], xT[0:P, 0:P])
            nc.gpsimd.dma_start(x_tiles[0][:, P:NBLK], xT[0:P, P:NBLK])
            for t in range(1, KT):
                nc.gpsimd.dma_start(
                    x_tiles[t][:, 0:NBLK], xT[t * P:(t + 1) * P, 0:NBLK]
                )
            for t in range(KT):
                nc.gpsimd.dma_start(
                    x_tiles[t][:, NBLK:2 * NBLK],
                    xT[t * P:(t + 1) * P, NBLK:2 * NBLK],
                )
            for t in range(KT):
                nc.gpsimd.dma_start(
                    x_tiles[t][:, 2 * NBLK:N_SHARD],
                    xT[t * P:(t + 1) * P, 2 * NBLK:N_SHARD],
                )

            # bias broadcast on the scalar sequencer.
            nc.scalar.dma_start(bias_sb[:], bias[:].to_broadcast((P, D_OUT)))

            # Leading warm-up matmuls (zeroed fp16 scratch, no data deps):
            # cold matmuls occupy the PE before the first real operands land,
            # pre-paying the p-state clock ramp. Memsets on vector (gpsimd is
            # busy issuing x descriptors).
            warm_x = const_pool.tile([P, P], bf16)
            warm_w = const_pool.tile([P, OH], bf16)
            nc.vector.memset(warm_x[:], 0.0)
            nc.vector.memset(warm_w[:], 0.0)
            warm_ps = psum_pool.tile([P, OH], f32, name="warm_ps", tag="psum")
            for _ in range(6):
                nc.tensor.matmul(warm_ps[:], warm_x[:], warm_w[:],
                                 start=True, stop=True)

            def evict(g, i, psum, halves=False):
                n0 = g * NBLK + i * P
                o_sb = out_pool.tile([P, D_OUT], bf16)
                if halves:
                    for h in range(2):
                        sl = slice(h * OH, (h + 1) * OH)
                        nc.vector.tensor_add(o_sb[:, sl], psum[:, sl],
                                             bias_sb[:, sl])
                        nc.scalar.dma_start(out[n0:n0 + P, sl], o_sb[:, sl])
                else:
                    nc.vector.tensor_add(o_sb[:], psum[:], bias_sb[:])
                    nc.scalar.dma_start(out[n0:n0 + P, :], o_sb[:])

            for g in range(NGRP):
                xt = [
                    x_tiles[t][:, g * NBLK:(g + 1) * NBLK]
                    for t in range(KT)
                ]
                psums = [
                    psum_pool.tile([P, D_OUT], f32, name=f"ps_g{g}_{i}",
                                   tag="psum")
                    for i in range(GRP)
                ]
                if g < NGRP - 1:
                    # d-outer: each arriving (w[d], x[d]) slice immediately
                    # enables 8 matmuls while later slices are in flight.
                    for d in range(KT):
                        for i in range(GRP):
                            lhsT = xt[d][:, i * P:(i + 1) * P]
                            for h in range(D_OUT // OH):
                                nc.tensor.matmul(
                                    psums[i][:, h * OH:(h + 1) * OH],
                                    lhsT,
                                    w_tiles[d][:, h * OH:(h + 1) * OH],
                                    start=(d == 0),
                                    stop=(d == KT - 1),
                                )
                    for i in range(GRP):
                        evict(g, i, psums[i])
                else:
                    # last group, data resident: i-outer spreads psum
                    # completions so the tail isn't 4 serialized evictions.
                    for i in range(GRP):
                        for d in range(KT):
                            lhsT = xt[d][:, i * P:(i + 1) * P]
                            for h in range(D_OUT // OH):
                                nc.tensor.matmul(
                                    psums[i][:, h * OH:(h + 1) * OH],
                                    lhsT,
                                    w_tiles[d][:, h * OH:(h + 1) * OH],
                                    start=(d == 0),
                                    stop=(d == KT - 1),
                                )
                        evict(g, i, psums[i], halves=True)

            # Trailing dummy matmuls: keep the PE busy through the final
            # eviction + output-DMA drain so the NC clock stays at full rate
            # into the wrapper epilogue (it halves ~2.5us after PE idle).
            for _ in range(10):
                nc.tensor.matmul(warm_ps[:], warm_x[:], warm_w[:],
                                 start=True, stop=True)

    nc.finalize()
    return nc


def _get_nc():
    if "nc" not in _CACHE:
        _CACHE["nc"] = build_nc()
    return _CACHE["nc"]


def kernel(x, weight, bias, A, B):
    x = np.asarray(x, dtype=np.float32)
    weight = np.asarray(weight, dtype=np.float32)
    bias = np.asarray(bias, dtype=np.float32)
    A = np.asarray(A, dtype=np.float32)
    B = np.asarray(B, dtype=np.float32)

    # Fold the rank-8 LoRA update into the weight (exact up to fp32 rounding).
    w_eff = (
        weight.astype(np.float64) + SCALING * (B.astype(np.float64) @ A.astype(np.float64))
    ).astype(np.float32)
    wT = np.ascontiguousarray(w_eff.T)  # [d, o] f32 (device reads as f32r)
    xT = np.ascontiguousarray(x.T)  # [d, n] f32 (device reads as f32r)
    bias2d = np.ascontiguousarray(bias.reshape(1, D_OUT))

    nc = _get_nc()
    in_maps = [
        {
            "xT": np.ascontiguousarray(xT[:, c * N_SHARD:(c + 1) * N_SHARD]),
            "wT": wT,
            "bias": bias2d,
        }
        for c in range(N_CORES)
    ]
    trace_kwargs = {}
    if os.environ.get("KERNEL_TRACE") == "1":
        trace_kwargs = {"trace": True}
    res = run_bass_kernel_spmd(nc, in_maps, list(range(N_CORES)), **trace_kwargs)
    _CACHE["last_results"] = res
    return np.concatenate(
        [r["out"].astype(np.float32) for r in res.results], axis=0
    )


# revision 9
# speedup vs baseline: 1.0793x; 1.0557x over previous
"""LoRA linear (y = x @ (W + s*B@A)^T + bias) on 8 Trainium2 NeuronCores.

Strategy: pure data parallel over the token dim. The LoRA update is folded
into the weight on the host (W' = W + 4.0 * B @ A, rank-8 update), so the
device kernel is a plain linear.

Measured facts this schedule is built around (from NTFF profiles):
  - 512-col matmul cadence: 227ns with an f32r moving operand vs 259ns
    with bf16 moving. So W (the moving operand) stays f32r; x (the
    stationary operand) must match width class (BIR verifier rejects
    32-bit x 16-bit mixes), so x stays f32r too.
  - DMA descriptor issue costs ~650ns *per dma_start on the issuing
    engine*; the old all-on-sync fill was descriptor-issue-bound (17
    descriptors ~ 11us). Descriptors are now spread: W on sync, x on
    gpsimd, bias on scalar, outputs on vector (right after each eviction,
    same-engine ordering for free).
  - Output is written bf16 (host upcasts), halving output traffic and the
    end-of-kernel write burst.
  - The NC clock drops to half rate ~2.5us after the PE goes idle and the
    fixed ~285-instruction wrapper epilogue then runs at half speed;
    trailing dummy matmuls keep the PE (and clock) busy through the
    eviction/DMA drain.

Per core: out[2048, 1024] = xT[:, shard].T @ wT + bias
  - wT [1024(d), 1024(o)] f32r resident in SBUF (4 MiB), loaded once
  - x resident as 8 d-tiles [128, 2048] f32r (8 MiB)
  - psum [128(n), 1024(o)] accumulated over 8 d-tiles, 2 o-halves of 512;
    4 psum tiles (all 8 banks) accumulate side by side so each arriving
    (w[d], x[d]) slice enables 8 matmuls during the fill phase
  - DVE adds bias (broadcast into [128, 1024] SBUF once) on PSUM eviction,
    writing bf16
"""

import os
import sys

import numpy as np

for _p in ("/opt/trn_rl_repo", "/opt/pypackages"):
    if os.path.isdir(_p) and _p not in sys.path:
        sys.path.append(_p)

import ml_dtypes  # noqa: E402,F401

try:
    import jax

    jax.config.update(
        "jax_compilation_cache_dir", os.path.expanduser("~/.cache/jax_bass_cache")
    )
    jax.config.update("jax_persistent_cache_min_compile_time_secs", 0.0)
except Exception:
    pass

try:
    # bass_utils imports this when tracing is requested via BASS_TRACE; the
    # agent image ships a stub antenv without it. Register a no-op fallback
    # so a trace request degrades to "no trace" instead of crashing.
    from antenv import axon_hooks as _axon_hooks  # noqa: F401
except ImportError:
    import types as _types

    import antenv as _antenv

    _hooks = _types.ModuleType("antenv.axon_hooks")
    _hooks._hook = None
    _hooks.set_axon_ntff_profile_hook = lambda h: setattr(_hooks, "_hook", h)
    _hooks.get_axon_ntff_profile_hook = lambda: _hooks._hook
    sys.modules["antenv.axon_hooks"] = _hooks
    _antenv.axon_hooks = _hooks

import concourse.bass as bass  # noqa: E402,F401
import concourse.mybir as mybir  # noqa: E402
import concourse.tile as tile  # noqa: E402
from concourse import bacc  # noqa: E402
from concourse.bass_utils import run_bass_kernel_spmd  # noqa: E402

N_CORES = 8
N_TOK, D_IN, D_OUT = 16384, 1024, 1024
N_SHARD = N_TOK // N_CORES  # 2048 tokens per core
P = 128
SCALING = 4.0  # alpha / r = 32 / 8

_CACHE: dict = {}


def build_nc():
    f32 = mybir.dt.float32
    f32r = mybir.dt.float32r
    f16 = mybir.dt.float16
    bf16 = mybir.dt.bfloat16
    nc = bacc.Bacc("TRN2", target_bir_lowering=False, debug=False)

    xT = nc.dram_tensor("xT", [D_IN, N_SHARD], f32r, kind="ExternalInput")
    wT = nc.dram_tensor("wT", [D_IN, D_OUT], f32r, kind="ExternalInput")
    bias = nc.dram_tensor("bias", [1, D_OUT], f32, kind="ExternalInput")
    out = nc.dram_tensor("out", [N_SHARD, D_OUT], bf16, kind="ExternalOutput")

    KT = D_IN // P  # 8 contraction tiles
    NBLK = 512  # tokens per group (4 psum tiles of 128)
    GRP = NBLK // P  # 4 psum tiles accumulated concurrently (8 banks)
    OH = 512  # max fp32 moving free dim (one PSUM bank)

    NGRP = N_SHARD // NBLK
    with tile.TileContext(nc) as tc:
        with tc.tile_pool(name="const", bufs=1) as const_pool, \
                tc.tile_pool(name="op", bufs=8) as out_pool, \
                tc.tile_pool(name="ps", bufs=GRP, space="PSUM") as psum_pool:
            w_tiles = [
                const_pool.tile([P, D_OUT], f32r, name=f"w{t}")
                for t in range(KT)
            ]
            x_tiles = [
                const_pool.tile([P, N_SHARD], f32r, name=f"x{t}")
                for t in range(KT)
            ]
            bias_sb = const_pool.tile([P, D_OUT], f32)

            # W as o-half descriptors (256KB each) so a matmul's first
            # half-tile dependency clears after half the bytes. d0-d3 issue
            # on sync, d4-d7 on scalar: two descriptor queues fill W in
            # parallel (descriptor issue costs ~650ns each on the engine).
            for t in range(KT // 2):
                for h in range(2):
                    nc.sync.dma_start(
                        w_tiles[t][:, h * OH:(h + 1) * OH],
                        wT[t * P:(t + 1) * P, h * OH:(h + 1) * OH],
                    )
            for t in range(KT // 2, KT):
                for h in range(2):
                    nc.scalar.dma_start(
                        w_tiles[t][:, h * OH:(h + 1) * OH],
                        wT[t * P:(t + 1) * P, h * OH:(h + 1) * OH],
                    )

            # Warm-up scratch memsets lead the gpsimd queue (2 x ~190ns)
            # so the PE's dependency-free warm matmuls start immediately
            # after the preamble, keeping the clock ramp continuous.
            warm_x = const_pool.tile([P, P], bf16)
            warm_w = const_pool.tile([P, OH], bf16)
            nc.gpsimd.memset(warm_x[:], 0.0)
            nc.gpsimd.memset(warm_w[:], 0.0)

            # x descriptors on the gpsimd sequencer, concurrently with W:
            # group-0 token slices first (fine-grained), then the remainder.
            nc.gpsimd.dma_start(x_tiles[0][:, 0:P], xT[0:P, 0:P])
            nc.gpsimd.dma_start(x_tiles[0][:, P:NBLK], xT[0:P, P:NBLK])
            for t in range(1, KT):
                nc.gpsimd.dma_start(
                    x_tiles[t][:, 0:NBLK], xT[t * P:(t + 1) * P, 0:NBLK]
                )
            for t in range(KT):
                nc.gpsimd.dma_start(
                    x_tiles[t][:, NBLK:2 * NBLK],
                    xT[t * P:(t + 1) * P, NBLK:2 * NBLK],
                )
            for t in range(KT):
                nc.gpsimd.dma_start(
                    x_tiles[t][:, 2 * NBLK:N_SHARD],
                    xT[t * P:(t + 1) * P, 2 * NBLK:N_SHARD],
                )

            # bias broadcast rides sync after its W descriptors.
            nc.sync.dma_start(bias_sb[:], bias[:].to_broadcast((P, D_OUT)))

            # Leading warm-up matmuls (zeroed bf16 scratch, no data deps):
            # cold matmuls occupy the PE before the first real operands land,
            # pre-paying the p-state clock ramp.
            warm_ps = psum_pool.tile([P, OH], f32, name="warm_ps", tag="psum")
            for _ in range(6):
                nc.tensor.matmul(warm_ps[:], warm_x[:], warm_w[:],
                                 start=True, stop=True)

            def evict(g, i, psum, halves=False):
                n0 = g * NBLK + i * P
                o_sb = out_pool.tile([P, D_OUT], bf16)
                if halves:
                    # final group: finer chunks shorten the serial tail
                    # after the last matmul (GPSIMD cannot read PSUM, so
                    # DVE does all of them).
                    nch = 4 if i == GRP - 1 else 2
                    cw = D_OUT // nch
                    for h in range(nch):
                        sl = slice(h * cw, (h + 1) * cw)
                        nc.vector.tensor_add(o_sb[:, sl], psum[:, sl],
                                             bias_sb[:, sl])
                        nc.sync.dma_start(out[n0:n0 + P, sl], o_sb[:, sl])
                else:
                    nc.vector.tensor_add(o_sb[:], psum[:], bias_sb[:])
                    nc.sync.dma_start(out[n0:n0 + P, :], o_sb[:])

            for g in range(NGRP):
                xt = [
                    x_tiles[t][:, g * NBLK:(g + 1) * NBLK]
                    for t in range(KT)
                ]
                psums = [
                    psum_pool.tile([P, D_OUT], f32, name=f"ps_g{g}_{i}",
                                   tag="psum")
                    for i in range(GRP)
                ]
                if g < NGRP - 1:
                    # d-outer: each arriving (w[d], x[d]) slice immediately
                    # enables 8 matmuls while later slices are in flight.
                    for d in range(KT):
                        for i in range(GRP):
                            lhsT = xt[d][:, i * P:(i + 1) * P]
                            for h in range(D_OUT // OH):
                                nc.tensor.matmul(
                                    psums[i][:, h * OH:(h + 1) * OH],
                                    lhsT,
                                    w_tiles[d][:, h * OH:(h + 1) * OH],
                                    start=(d == 0),
                                    stop=(d == KT - 1),
                                )
                    for i in range(GRP):
                        evict(g, i, psums[i])
                else:
                    # last group, data resident: i-outer spreads psum
                    # completions so the tail isn't 4 serialized evictions.
                    for i in range(GRP):
                        for d in range(KT):
                            lhsT = xt[d][:, i * P:(i + 1) * P]
                            for h in range(D_OUT // OH):
                                nc.tensor.matmul(
                                    psums[i][:, h * OH:(h + 1) * OH],
                                    lhsT,
                                    w_tiles[d][:, h * OH:(h + 1) * OH],
                                    start=(d == 0),
                                    stop=(d == KT - 1),
                                )
                        evict(g, i, psums[i], halves=True)

            # Trailing dummy matmuls: keep the PE busy through the final
            # eviction + output-DMA drain so the NC clock stays at full rate
            # into the wrapper epilogue (it halves ~2.5us after PE idle).
            for _ in range(14):
                nc.tensor.matmul(warm_ps[:], warm_x[:], warm_w[:],
                                 start=True, stop=True)

    nc.finalize()
    return nc


def _get_nc():
    if "nc" not in _CACHE:
        _CACHE["nc"] = build_nc()
    return _CACHE["nc"]


def kernel(x, weight, bias, A, B):
    x = np.asarray(x, dtype=np.float32)
    weight = np.asarray(weight, dtype=np.float32)
    bias = np.asarray(bias, dtype=np.float32)
    A = np.asarray(A, dtype=np.float32)
    B = np.asarray(B, dtype=np.float32)

    # Fold the rank-8 LoRA update into the weight (exact up to fp32 rounding).
    w_eff = (
        weight.astype(np.float64) + SCALING * (B.astype(np.float64) @ A.astype(np.float64))
    ).astype(np.float32)
    wT = np.ascontiguousarray(w_eff.T)  # [d, o] f32 (device reads as f32r)
    xT = np.ascontiguousarray(x.T)  # [d, n] f32 (device reads as f32r)
    bias2d = np.ascontiguousarray(bias.reshape(1, D_OUT))

    nc = _get_nc()
    in_maps = [
        {
            "xT": np.ascontiguousarray(xT[:, c * N_SHARD:(c + 1) * N_SHARD]),
            "wT": wT,
            "bias": bias2d,
        }
        for c in range(N_CORES)
    ]
    trace_kwargs = {}
    if os.environ.get("KERNEL_TRACE") == "1":
        trace_kwargs = {"trace": True}
    res = run_bass_kernel_spmd(nc, in_maps, list(range(N_CORES)), **trace_kwargs)
    _CACHE["last_results"] = res
    return np.concatenate(
        [r["out"].astype(np.float32) for r in res.results], axis=0
    )
